# revision 3
# baseline (speedup 1.0000x reference)
"""Expert-choice MoE FFN kernel for Trainium2 (8 NeuronCores, expert-parallel).

Strategy (per sharding hint): expert parallelism — one expert per core.
- Host: router scores + expert-choice top-k + softmax (exactly mirrors the
  reference's jax ops on CPU so the selected token SET is bit-identical),
  gather of chosen tokens (transposed to D-major for the PE), and the final
  index_add scatter of the 8 per-expert contributions.
- Device (per core e): SwiGLU FFN over that expert's 2048 chosen tokens:
    gateT/upT = Wg[e].T-chunks @ chosenT, hid = silu(gate)*up  (F-major)
    eout = hidT-chunks @ Wd[e], scaled by per-token softmax weight.
  Matmuls run in float32r (fp32 with 11-bit mantissa, 4x faster than fp32
  on the PE at free-dim >= 256), accumulation in fp32 PSUM.

Self-contained: shapes hardcoded from the problem spec.
"""

import math

import numpy as np

# Problem shapes (hardcoded per contract)
D = 1024          # d_model
F = 4096          # d_ff
E = 8             # experts
TOP_K = 2
B, S = 4, 2048
N = B * S         # 8192 tokens
CAP = min(math.ceil(N * TOP_K / E), N)  # 2048
N_CORES = 8

KD = D // 128     # 8  K-chunks over D
KF = F // 128     # 32 K-chunks over F
CC_SIZE = 1024    # cap-chunk (tokens processed per outer iteration)
N_CC = CAP // CC_SIZE          # 2
FH_KF = KF // 2                # 16 f-chunks per F-half
W_SLAB = 256                   # f-width of streamed Wg/Wu slabs
D_CH = 256                     # d-width of phase-B output chunks

_CACHE = {}


def _build_module():
    import concourse.tile as tile
    from concourse import bacc, mybir

    F32 = mybir.dt.float32
    F32R = mybir.dt.float32r
    AF = mybir.ActivationFunctionType

    nc = bacc.Bacc("TRN2", target_bir_lowering=False, debug=False)

    xt = nc.dram_tensor("xt", [D, CAP], F32R, kind="ExternalInput").ap()
    wg = nc.dram_tensor("wg", [D, F], F32R, kind="ExternalInput").ap()
    wu = nc.dram_tensor("wu", [D, F], F32R, kind="ExternalInput").ap()
    wd = nc.dram_tensor("wd", [F, D], F32R, kind="ExternalInput").ap()
    wt = nc.dram_tensor("wt", [128, CAP // 128], F32, kind="ExternalInput").ap()
    contrib = nc.dram_tensor("contrib", [CAP, D], F32, kind="ExternalOutput").ap()

    xt_r = xt.rearrange("(ko p) c -> p ko c", p=128)        # [128, 8, 2048]
    wg_r = wg.rearrange("(ko p) f -> p ko f", p=128)        # [128, 8, 4096]
    wu_r = wu.rearrange("(ko p) f -> p ko f", p=128)
    wd_r = wd.rearrange("(ko p) d -> p ko d", p=128)        # [128, 32, 1024]
    c_r = contrib.rearrange("(mo p) d -> p mo d", p=128)    # [128, 16, 1024]

    with tile.TileContext(nc) as tc:
        with (
            tc.tile_pool(name="xp", bufs=1) as xpool,
            tc.tile_pool(name="wp", bufs=2) as wpool,
            tc.tile_pool(name="hp", bufs=1) as hpool,
            tc.tile_pool(name="ep", bufs=1) as epool,
            tc.tile_pool(name="sp", bufs=3) as spool,
            tc.tile_pool(name="op", bufs=2) as opool,
            tc.tile_pool(name="cp", bufs=1) as cpool,
            tc.tile_pool(name="ps", bufs=2, space="PSUM") as pspool,
        ):
            wt_t = cpool.tile([128, CAP // 128], F32)
            nc.sync.dma_start(wt_t[:], wt)

            for cc in range(N_CC):
                xt_t = xpool.tile([128, KD, CC_SIZE], F32R, tag="xt")
                nc.sync.dma_start(
                    xt_t[:], xt_r[:, :, cc * CC_SIZE : (cc + 1) * CC_SIZE]
                )
                eout = epool.tile([128, CC_SIZE // 128, D], F32, tag="eout")

                for fh in range(2):
                    # ---- Phase A: gateT/upT chunks -> hid (F-major) ----
                    hid = hpool.tile([128, FH_KF, CC_SIZE], F32R, tag="hid")
                    n_slabs = FH_KF * 128 // W_SLAB          # 8
                    for fs in range(n_slabs):
                        f0 = fh * (FH_KF * 128) + fs * W_SLAB
                        wg_s = wpool.tile([128, KD, W_SLAB], F32R, tag="wg")
                        wu_s = wpool.tile([128, KD, W_SLAB], F32R, tag="wu")
                        nc.sync.dma_start(wg_s[:], wg_r[:, :, f0 : f0 + W_SLAB])
                        nc.sync.dma_start(wu_s[:], wu_r[:, :, f0 : f0 + W_SLAB])
                        for fi in range(W_SLAB // 128):      # 2
                            kf_loc = fs * (W_SLAB // 128) + fi
                            for n in range(CC_SIZE // 512):  # 2
                                pg = pspool.tile([128, 512], F32, tag="pg")
                                pu = pspool.tile([128, 512], F32, tag="pu")
                                for k in range(KD):
                                    nc.tensor.matmul(
                                        pg[:],
                                        lhsT=wg_s[:, k, fi * 128 : (fi + 1) * 128],
                                        rhs=xt_t[:, k, n * 512 : (n + 1) * 512],
                                        start=(k == 0),
                                        stop=(k == KD - 1),
                                    )
                                for k in range(KD):
                                    nc.tensor.matmul(
                                        pu[:],
                                        lhsT=wu_s[:, k, fi * 128 : (fi + 1) * 128],
                                        rhs=xt_t[:, k, n * 512 : (n + 1) * 512],
                                        start=(k == 0),
                                        stop=(k == KD - 1),
                                    )
                                sg = spool.tile([128, 512], F32, tag="sg")
                                nc.scalar.activation(sg[:], pg[:], AF.Silu)
                                nc.vector.tensor_mul(
                                    hid[:, kf_loc, n * 512 : (n + 1) * 512],
                                    sg[:],
                                    pu[:],
                                )

                    # ---- Phase B (partial over this F-half): eout += hidT @ Wd ----
                    for d in range(D // D_CH):               # 4
                        wd_s = wpool.tile([128, FH_KF, D_CH], F32R, tag="wd")
                        nc.sync.dma_start(
                            wd_s[:],
                            wd_r[:, fh * FH_KF : (fh + 1) * FH_KF, d * D_CH : (d + 1) * D_CH],
                        )
                        for m in range(CC_SIZE // 128):      # 8
                            po = pspool.tile([128, D_CH], F32, tag="po")
                            for kf in range(FH_KF):
                                nc.tensor.matmul(
                                    po[:],
                                    lhsT=hid[:, kf, m * 128 : (m + 1) * 128],
                                    rhs=wd_s[:, kf, :],
                                    start=(kf == 0),
                                    stop=(kf == FH_KF - 1),
                                )
                            dsl = slice(d * D_CH, (d + 1) * D_CH)
                            if fh == 0:
                                nc.vector.tensor_copy(eout[:, m, dsl], po[:])
                            else:
                                nc.vector.tensor_add(
                                    eout[:, m, dsl], eout[:, m, dsl], po[:]
                                )

                # ---- scale by per-token router weight, store ----
                for m in range(CC_SIZE // 128):
                    gm = cc * (CC_SIZE // 128) + m
                    ot = opool.tile([128, D], F32, tag="ot")
                    nc.scalar.activation(
                        ot[:], eout[:, m], AF.Copy, scale=wt_t[:, gm : gm + 1]
                    )
                    nc.sync.dma_start(c_r[:, gm, :], ot[:])

    nc.compile()
    return nc


def _get_module():
    if "nc" not in _CACHE:
        _CACHE["nc"] = _build_module()
    return _CACHE["nc"]


def _route_host(x_flat, Wr):
    """Mirror the reference's routing exactly (jax ops on CPU)."""
    import jax
    import jax.numpy as jnp

    cpu = jax.devices("cpu")[0]
    with jax.default_device(cpu):
        xj = jnp.asarray(np.asarray(x_flat, dtype=np.float32))
        wj = jnp.asarray(np.asarray(Wr, dtype=np.float32))
        scores = (xj @ wj.T).T                      # (E, N)
        top_vals, top_idx = jax.lax.top_k(scores, CAP)
        weights = jax.nn.softmax(top_vals, axis=-1)
        return np.asarray(top_idx), np.asarray(weights)


def kernel(x, Wr, Wg, Wu, Wd):
    from concourse.bass_utils import run_bass_kernel_spmd

    x = np.asarray(x, dtype=np.float32)
    Wr = np.asarray(Wr, dtype=np.float32)
    Wg = np.asarray(Wg, dtype=np.float32)
    Wu = np.asarray(Wu, dtype=np.float32)
    Wd = np.asarray(Wd, dtype=np.float32)

    x_flat = x.reshape(N, D)
    top_idx, weights = _route_host(x_flat, Wr)

    nc = _get_module()
    in_maps = []
    for e in range(N_CORES):
        chosenT = np.ascontiguousarray(x_flat[top_idx[e]].T)            # (D, CAP)
        wt_tiled = np.ascontiguousarray(
            weights[e].reshape(CAP // 128, 128).T                        # (128, 16)
        )
        in_maps.append(
            {
                "xt": chosenT,
                "wg": np.ascontiguousarray(Wg[e]),
                "wu": np.ascontiguousarray(Wu[e]),
                "wd": np.ascontiguousarray(Wd[e]),
                "wt": wt_tiled,
            }
        )

    _CACHE["last_in_maps"] = in_maps
    res = run_bass_kernel_spmd(nc, in_maps, core_ids=list(range(N_CORES)))
    _CACHE["last_results"] = res

    out = np.zeros((N, D), dtype=np.float32)
    for e in range(N_CORES):
        out[top_idx[e]] += res.results[e]["contrib"]
    aux_loss = np.asarray(0.0, dtype=np.float32)
    return out.reshape(B, S, D), aux_loss


# revision 5
# speedup vs baseline: 102.7769x; 102.7769x over previous
"""Expert-choice MoE FFN kernel for Trainium2 (8 NeuronCores, expert-parallel).

Strategy (per sharding hint): expert parallelism — one expert per core.
- Host: router scores + expert-choice top-k + softmax (exactly mirrors the
  reference's jax ops on CPU so the selected token SET is bit-identical),
  gather of chosen tokens (transposed to D-major for the PE), and the final
  index_add scatter of the 8 per-expert contributions.
- Device (per core e): SwiGLU FFN over that expert's 2048 chosen tokens:
    gateT/upT = Wg[e].T-chunks @ chosenT, hid = silu(gate)*up  (F-major)
    eout = hidT-chunks @ Wd[e], scaled by per-token softmax weight.
  Matmuls run in float32r (fp32 with 11-bit mantissa, 4x faster than fp32
  on the PE at free-dim >= 256), accumulation in fp32 PSUM.

Self-contained: shapes hardcoded from the problem spec.
"""

import math

import numpy as np

# Problem shapes (hardcoded per contract)
D = 1024          # d_model
F = 4096          # d_ff
E = 8             # experts
TOP_K = 2
B, S = 4, 2048
N = B * S         # 8192 tokens
CAP = min(math.ceil(N * TOP_K / E), N)  # 2048
N_CORES = 8

KD = D // 128     # 8  K-chunks over D
KF = F // 128     # 32 K-chunks over F
CC_SIZE = 1024    # cap-chunk (tokens processed per outer iteration)
N_CC = CAP // CC_SIZE          # 2
FH_KF = KF // 2                # 16 f-chunks per F-half
W_SLAB = 256                   # f-width of streamed Wg/Wu slabs
D_CH = 256                     # d-width of phase-B output chunks

_CACHE = {}


def _build_module(n_repeat=1):
    import concourse.tile as tile
    from concourse import bacc, mybir

    F32 = mybir.dt.float32
    F32R = mybir.dt.float32r
    AF = mybir.ActivationFunctionType

    nc = bacc.Bacc("TRN2", target_bir_lowering=False, debug=False)

    xt = nc.dram_tensor("xt", [D, CAP], F32R, kind="ExternalInput").ap()
    wg = nc.dram_tensor("wg", [D, F], F32R, kind="ExternalInput").ap()
    wu = nc.dram_tensor("wu", [D, F], F32R, kind="ExternalInput").ap()
    wd = nc.dram_tensor("wd", [F, D], F32R, kind="ExternalInput").ap()
    wt = nc.dram_tensor("wt", [128, CAP // 128], F32, kind="ExternalInput").ap()
    contrib = nc.dram_tensor("contrib", [CAP, D], F32, kind="ExternalOutput").ap()

    xt_r = xt.rearrange("(ko p) c -> p ko c", p=128)        # [128, 8, 2048]
    wg_r = wg.rearrange("(ko p) f -> p ko f", p=128)        # [128, 8, 4096]
    wu_r = wu.rearrange("(ko p) f -> p ko f", p=128)
    wd_r = wd.rearrange("(ko p) d -> p ko d", p=128)        # [128, 32, 1024]
    c_r = contrib.rearrange("(mo p) d -> p mo d", p=128)    # [128, 16, 1024]

    with tile.TileContext(nc) as tc:
        with (
            tc.tile_pool(name="xp", bufs=1) as xpool,
            tc.tile_pool(name="wp", bufs=2) as wpool,
            tc.tile_pool(name="hp", bufs=1) as hpool,
            tc.tile_pool(name="ep", bufs=1) as epool,
            tc.tile_pool(name="sp", bufs=3) as spool,
            tc.tile_pool(name="op", bufs=2) as opool,
            tc.tile_pool(name="cp", bufs=1) as cpool,
            tc.tile_pool(name="ps", bufs=2, space="PSUM") as pspool,
        ):
            wt_t = cpool.tile([128, CAP // 128], F32)
            nc.sync.dma_start(wt_t[:], wt)

            def _emit_body():
                for cc in range(N_CC):
                    xt_t = xpool.tile([128, KD, CC_SIZE], F32R, tag="xt")
                    nc.sync.dma_start(
                        xt_t[:], xt_r[:, :, cc * CC_SIZE : (cc + 1) * CC_SIZE]
                    )
                    eout = epool.tile([128, CC_SIZE // 128, D], F32, tag="eout")

                    for fh in range(2):
                        # ---- Phase A: gateT/upT chunks -> hid (F-major) ----
                        hid = hpool.tile([128, FH_KF, CC_SIZE], F32R, tag="hid")
                        n_slabs = FH_KF * 128 // W_SLAB          # 8
                        for fs in range(n_slabs):
                            f0 = fh * (FH_KF * 128) + fs * W_SLAB
                            wg_s = wpool.tile([128, KD, W_SLAB], F32R, tag="wg")
                            wu_s = wpool.tile([128, KD, W_SLAB], F32R, tag="wu")
                            nc.sync.dma_start(wg_s[:], wg_r[:, :, f0 : f0 + W_SLAB])
                            nc.sync.dma_start(wu_s[:], wu_r[:, :, f0 : f0 + W_SLAB])
                            for fi in range(W_SLAB // 128):      # 2
                                kf_loc = fs * (W_SLAB // 128) + fi
                                for n in range(CC_SIZE // 512):  # 2
                                    pg = pspool.tile([128, 512], F32, tag="pg")
                                    pu = pspool.tile([128, 512], F32, tag="pu")
                                    for k in range(KD):
                                        nc.tensor.matmul(
                                            pg[:],
                                            lhsT=wg_s[:, k, fi * 128 : (fi + 1) * 128],
                                            rhs=xt_t[:, k, n * 512 : (n + 1) * 512],
                                            start=(k == 0),
                                            stop=(k == KD - 1),
                                        )
                                    for k in range(KD):
                                        nc.tensor.matmul(
                                            pu[:],
                                            lhsT=wu_s[:, k, fi * 128 : (fi + 1) * 128],
                                            rhs=xt_t[:, k, n * 512 : (n + 1) * 512],
                                            start=(k == 0),
                                            stop=(k == KD - 1),
                                        )
                                    sg = spool.tile([128, 512], F32, tag="sg")
                                    nc.scalar.activation(sg[:], pg[:], AF.Silu)
                                    nc.vector.tensor_mul(
                                        hid[:, kf_loc, n * 512 : (n + 1) * 512],
                                        sg[:],
                                        pu[:],
                                    )

                        # ---- Phase B (partial): eout += hidT @ Wd ----
                        for d in range(D // D_CH):               # 4
                            wd_s = wpool.tile([128, FH_KF, D_CH], F32R, tag="wd")
                            nc.sync.dma_start(
                                wd_s[:],
                                wd_r[
                                    :,
                                    fh * FH_KF : (fh + 1) * FH_KF,
                                    d * D_CH : (d + 1) * D_CH,
                                ],
                            )
                            for m in range(CC_SIZE // 128):      # 8
                                po = pspool.tile([128, D_CH], F32, tag="po")
                                for kf in range(FH_KF):
                                    nc.tensor.matmul(
                                        po[:],
                                        lhsT=hid[:, kf, m * 128 : (m + 1) * 128],
                                        rhs=wd_s[:, kf, :],
                                        start=(kf == 0),
                                        stop=(kf == FH_KF - 1),
                                    )
                                dsl = slice(d * D_CH, (d + 1) * D_CH)
                                if fh == 0:
                                    nc.vector.tensor_copy(eout[:, m, dsl], po[:])
                                else:
                                    nc.vector.tensor_add(
                                        eout[:, m, dsl], eout[:, m, dsl], po[:]
                                    )

                    # ---- scale by per-token router weight, store ----
                    for m in range(CC_SIZE // 128):
                        gm = cc * (CC_SIZE // 128) + m
                        ot = opool.tile([128, D], F32, tag="ot")
                        nc.scalar.activation(
                            ot[:], eout[:, m], AF.Copy, scale=wt_t[:, gm : gm + 1]
                        )
                        nc.sync.dma_start(c_r[:, gm, :], ot[:])

            if n_repeat == 1:
                _emit_body()
            else:
                with tc.For_i(0, n_repeat, 1):
                    _emit_body()

    nc.compile()
    return nc


def _get_module():
    if "nc" not in _CACHE:
        _CACHE["nc"] = _build_module()
    return _CACHE["nc"]


def _route_host(x_flat, Wr):
    """Mirror the reference's routing exactly (jax ops on CPU)."""
    import jax
    import jax.numpy as jnp

    cpu = jax.devices("cpu")[0]
    with jax.default_device(cpu):
        xj = jnp.asarray(np.asarray(x_flat, dtype=np.float32))
        wj = jnp.asarray(np.asarray(Wr, dtype=np.float32))
        scores = (xj @ wj.T).T                      # (E, N)
        top_vals, top_idx = jax.lax.top_k(scores, CAP)
        weights = jax.nn.softmax(top_vals, axis=-1)
        return np.asarray(top_idx), np.asarray(weights)


def kernel(x, Wr, Wg, Wu, Wd):
    from concourse.bass_utils import run_bass_kernel_spmd

    x = np.asarray(x, dtype=np.float32)
    Wr = np.asarray(Wr, dtype=np.float32)
    Wg = np.asarray(Wg, dtype=np.float32)
    Wu = np.asarray(Wu, dtype=np.float32)
    Wd = np.asarray(Wd, dtype=np.float32)

    x_flat = x.reshape(N, D)
    top_idx, weights = _route_host(x_flat, Wr)

    nc = _get_module()
    in_maps = []
    for e in range(N_CORES):
        chosenT = np.ascontiguousarray(x_flat[top_idx[e]].T)            # (D, CAP)
        wt_tiled = np.ascontiguousarray(
            weights[e].reshape(CAP // 128, 128).T                        # (128, 16)
        )
        in_maps.append(
            {
                "xt": chosenT,
                "wg": np.ascontiguousarray(Wg[e]),
                "wu": np.ascontiguousarray(Wu[e]),
                "wd": np.ascontiguousarray(Wd[e]),
                "wt": wt_tiled,
            }
        )

    _CACHE["last_in_maps"] = in_maps
    res = run_bass_kernel_spmd(nc, in_maps, core_ids=list(range(N_CORES)))
    _CACHE["last_results"] = res

    out = np.zeros((N, D), dtype=np.float32)
    for e in range(N_CORES):
        out[top_idx[e]] += res.results[e]["contrib"]
    aux_loss = np.asarray(0.0, dtype=np.float32)
    return out.reshape(B, S, D), aux_loss


# revision 11
# speedup vs baseline: 107.8108x; 1.0490x over previous
"""Expert-choice MoE FFN kernel for Trainium2 (8 NeuronCores, expert-parallel).

Strategy (per sharding hint): expert parallelism — one expert per core.
- Host: router scores + expert-choice top-k + softmax (exactly mirrors the
  reference's jax ops on CPU so the selected token SET is bit-identical),
  gather of chosen tokens (transposed to D-major for the PE), and the final
  index_add scatter of the 8 per-expert contributions.
- Device (per core e): SwiGLU FFN over that expert's 2048 chosen tokens:
    gateT/upT = Wg[e].T-chunks @ chosenT, hid = silu(gate)*up  (F-major)
    eout = hidT-chunks @ Wd[e], scaled by per-token softmax weight.
  Matmuls run in float32r (fp32 with 11-bit mantissa, 4x faster than fp32
  on the PE at free-dim >= 256), accumulation in fp32 PSUM.

Self-contained: shapes hardcoded from the problem spec.
"""

import math

import numpy as np

# Problem shapes (hardcoded per contract)
D = 1024          # d_model
F = 4096          # d_ff
E = 8             # experts
TOP_K = 2
B, S = 4, 2048
N = B * S         # 8192 tokens
CAP = min(math.ceil(N * TOP_K / E), N)  # 2048
N_CORES = 8

KD = D // 128     # 8  K-chunks over D
KF = F // 128     # 32 K-chunks over F
CC_SIZE = 1024    # cap-chunk (tokens processed per outer iteration)
N_CC = CAP // CC_SIZE          # 2
FH_KF = KF // 2                # 16 f-chunks per F-half
W_SLAB = 128                   # f-width of streamed Wg/Wu slabs
D_CH = 256                     # d-width of phase-B output chunks

_CACHE = {}


def _build_module(n_repeat=1):
    import concourse.tile as tile
    from concourse import bacc, mybir

    F32 = mybir.dt.float32
    F32R = mybir.dt.float32r
    AF = mybir.ActivationFunctionType

    nc = bacc.Bacc("TRN2", target_bir_lowering=False, debug=False)

    xt = nc.dram_tensor("xt", [D, CAP], F32R, kind="ExternalInput").ap()
    wg = nc.dram_tensor("wg", [D, F], F32R, kind="ExternalInput").ap()
    wu = nc.dram_tensor("wu", [D, F], F32R, kind="ExternalInput").ap()
    wd = nc.dram_tensor("wd", [F, D], F32R, kind="ExternalInput").ap()
    wt = nc.dram_tensor("wt", [128, CAP // 128], F32, kind="ExternalInput").ap()
    contrib = nc.dram_tensor("contrib", [CAP, D], F32, kind="ExternalOutput").ap()

    xt_r = xt.rearrange("(ko p) c -> p ko c", p=128)        # [128, 8, 2048]
    wg_r = wg.rearrange("(ko p) f -> p ko f", p=128)        # [128, 8, 4096]
    wu_r = wu.rearrange("(ko p) f -> p ko f", p=128)
    wd_r = wd.rearrange("(ko p) d -> p ko d", p=128)        # [128, 32, 1024]
    c_r = contrib.rearrange("(mo p) d -> p mo d", p=128)    # [128, 16, 1024]

    with tile.TileContext(nc) as tc:
        with (
            tc.tile_pool(name="xp", bufs=1) as xpool,
            tc.tile_pool(name="wp", bufs=2) as wpool,
            tc.tile_pool(name="hp", bufs=1) as hpool,
            tc.tile_pool(name="ep", bufs=1) as epool,
            tc.tile_pool(name="sp", bufs=2) as spool,
            tc.tile_pool(name="cp", bufs=1) as cpool,
            tc.tile_pool(name="ps", bufs=2, space="PSUM") as pspool,
        ):
            wt_t = cpool.tile([128, CAP // 128], F32)
            nc.sync.dma_start(wt_t[:], wt)

            def _emit_body():
                for cc in range(N_CC):
                    xt_t = xpool.tile([128, KD, CC_SIZE], F32R, tag="xt")
                    # split the load so the first matmuls only wait on the
                    # first half of the cap-chunk
                    for nh in range(2):
                        nsl = slice(nh * (CC_SIZE // 2), (nh + 1) * (CC_SIZE // 2))
                        nc.sync.dma_start(
                            xt_t[:, :, nsl],
                            xt_r[:, :, cc * CC_SIZE + nh * (CC_SIZE // 2) :
                                 cc * CC_SIZE + (nh + 1) * (CC_SIZE // 2)],
                        )
                    eout = epool.tile([128, CC_SIZE // 128, D], F32, tag="eout")

                    for fh in range(2):
                        # ---- Phase A: gateT/upT chunks -> hid (F-major) ----
                        hid = hpool.tile([128, FH_KF, CC_SIZE], F32R, tag="hid")
                        n_slabs = FH_KF * 128 // W_SLAB          # 8
                        for fs in range(n_slabs):
                            f0 = fh * (FH_KF * 128) + fs * W_SLAB
                            wg_s = wpool.tile([128, KD, W_SLAB], F32R, tag="wg")
                            wu_s = wpool.tile([128, KD, W_SLAB], F32R, tag="wu")
                            nc.sync.dma_start(wg_s[:], wg_r[:, :, f0 : f0 + W_SLAB])
                            nc.sync.dma_start(wu_s[:], wu_r[:, :, f0 : f0 + W_SLAB])
                            for fi in range(W_SLAB // 128):      # 2
                                kf_loc = fs * (W_SLAB // 128) + fi
                                for n in range(CC_SIZE // 512):  # 2
                                    pg = pspool.tile([128, 512], F32, tag="pg")
                                    pu = pspool.tile([128, 512], F32, tag="pu")
                                    for k in range(KD):
                                        nc.tensor.matmul(
                                            pg[:],
                                            lhsT=wg_s[:, k, fi * 128 : (fi + 1) * 128],
                                            rhs=xt_t[:, k, n * 512 : (n + 1) * 512],
                                            start=(k == 0),
                                            stop=(k == KD - 1),
                                        )
                                    for k in range(KD):
                                        nc.tensor.matmul(
                                            pu[:],
                                            lhsT=wu_s[:, k, fi * 128 : (fi + 1) * 128],
                                            rhs=xt_t[:, k, n * 512 : (n + 1) * 512],
                                            start=(k == 0),
                                            stop=(k == KD - 1),
                                        )
                                    sg = spool.tile([128, 512], F32, tag="sg")
                                    nc.scalar.activation(sg[:], pg[:], AF.Silu)
                                    nc.vector.tensor_mul(
                                        hid[:, kf_loc, n * 512 : (n + 1) * 512],
                                        sg[:],
                                        pu[:],
                                    )

                        # ---- Phase B (partial): eout += hidT @ Wd ----
                        for d in range(D // D_CH):               # 4
                            wd_s = wpool.tile([128, FH_KF, D_CH], F32R, tag="wd")
                            nc.sync.dma_start(
                                wd_s[:],
                                wd_r[
                                    :,
                                    fh * FH_KF : (fh + 1) * FH_KF,
                                    d * D_CH : (d + 1) * D_CH,
                                ],
                            )
                            for m in range(CC_SIZE // 128):      # 8
                                gm = cc * (CC_SIZE // 128) + m
                                po = pspool.tile([128, D_CH], F32, tag="po")
                                for kf in range(FH_KF):
                                    nc.tensor.matmul(
                                        po[:],
                                        lhsT=hid[:, kf, m * 128 : (m + 1) * 128],
                                        rhs=wd_s[:, kf, :],
                                        start=(kf == 0),
                                        stop=(kf == FH_KF - 1),
                                    )
                                dsl = slice(d * D_CH, (d + 1) * D_CH)
                                if fh == 0:
                                    # eout = po * w   (per-token router weight)
                                    nc.vector.tensor_scalar_mul(
                                        eout[:, m, dsl], po[:], wt_t[:, gm : gm + 1]
                                    )
                                else:
                                    # eout = (po * w) + eout, then store
                                    nc.vector.scalar_tensor_tensor(
                                        eout[:, m, dsl],
                                        po[:],
                                        wt_t[:, gm : gm + 1],
                                        eout[:, m, dsl],
                                        op0=mybir.AluOpType.mult,
                                        op1=mybir.AluOpType.add,
                                    )

                    # ---- store scaled outputs ----
                    for m in range(CC_SIZE // 128):
                        gm = cc * (CC_SIZE // 128) + m
                        nc.sync.dma_start(c_r[:, gm, :], eout[:, m, :])

            if n_repeat == 1:
                _emit_body()
            else:
                with tc.For_i(0, n_repeat, 1):
                    _emit_body()

    nc.compile()
    return nc


def _get_module():
    if "nc" not in _CACHE:
        _CACHE["nc"] = _build_module()
    return _CACHE["nc"]


def _route_host(x_flat, Wr):
    """Mirror the reference's routing exactly (jax ops on CPU)."""
    import jax
    import jax.numpy as jnp

    cpu = jax.devices("cpu")[0]
    with jax.default_device(cpu):
        xj = jnp.asarray(np.asarray(x_flat, dtype=np.float32))
        wj = jnp.asarray(np.asarray(Wr, dtype=np.float32))
        scores = (xj @ wj.T).T                      # (E, N)
        top_vals, top_idx = jax.lax.top_k(scores, CAP)
        weights = jax.nn.softmax(top_vals, axis=-1)
        return np.asarray(top_idx), np.asarray(weights)


def kernel(x, Wr, Wg, Wu, Wd):
    from concourse.bass_utils import run_bass_kernel_spmd

    x = np.asarray(x, dtype=np.float32)
    Wr = np.asarray(Wr, dtype=np.float32)
    Wg = np.asarray(Wg, dtype=np.float32)
    Wu = np.asarray(Wu, dtype=np.float32)
    Wd = np.asarray(Wd, dtype=np.float32)

    x_flat = x.reshape(N, D)
    top_idx, weights = _route_host(x_flat, Wr)

    nc = _get_module()
    in_maps = []
    for e in range(N_CORES):
        chosenT = np.ascontiguousarray(x_flat[top_idx[e]].T)            # (D, CAP)
        wt_tiled = np.ascontiguousarray(
            weights[e].reshape(CAP // 128, 128).T                        # (128, 16)
        )
        in_maps.append(
            {
                "xt": chosenT,
                "wg": np.ascontiguousarray(Wg[e]),
                "wu": np.ascontiguousarray(Wu[e]),
                "wd": np.ascontiguousarray(Wd[e]),
                "wt": wt_tiled,
            }
        )

    _CACHE["last_in_maps"] = in_maps
    res = run_bass_kernel_spmd(nc, in_maps, core_ids=list(range(N_CORES)))
    _CACHE["last_results"] = res

    out = np.zeros((N, D), dtype=np.float32)
    for e in range(N_CORES):
        out[top_idx[e]] += res.results[e]["contrib"]
    aux_loss = np.asarray(0.0, dtype=np.float32)
    return out.reshape(B, S, D), aux_loss


# revision 14
# speedup vs baseline: 127.3109x; 1.1809x over previous
"""Expert-choice MoE FFN kernel for Trainium2 (8 NeuronCores, expert-parallel).

Strategy (per sharding hint): expert parallelism — one expert per core.
- Host: router scores + expert-choice top-k + softmax (exactly mirrors the
  reference's jax ops on CPU so the selected token SET is bit-identical),
  gather of chosen tokens (transposed to D-major for the PE), and the final
  index_add scatter of the 8 per-expert contributions.
- Device (per core e): SwiGLU FFN over that expert's 2048 chosen tokens:
    gateT/upT = Wg[e].T-chunks @ chosenT, hid = silu(gate)*up  (F-major)
    eout = hidT-chunks @ Wd[e], scaled by per-token softmax weight.
  Matmuls run in float32r (fp32 with 11-bit mantissa, 4x faster than fp32
  on the PE at free-dim >= 256), accumulation in fp32 PSUM.

Self-contained: shapes hardcoded from the problem spec.
"""

import math

import numpy as np

# Problem shapes (hardcoded per contract)
D = 1024          # d_model
F = 4096          # d_ff
E = 8             # experts
TOP_K = 2
B, S = 4, 2048
N = B * S         # 8192 tokens
CAP = min(math.ceil(N * TOP_K / E), N)  # 2048
N_CORES = 8

KD = D // 128     # 8  K-chunks over D
KF = F // 128     # 32 K-chunks over F
CC_SIZE = 1024    # cap-chunk (tokens processed per outer iteration)
N_CC = CAP // CC_SIZE          # 2
N_FQ = 4                       # F processed in quarters
FQ_KF = KF // N_FQ             # 8 f-chunks per F-quarter
W_SLAB = 128                   # f-width of streamed Wg/Wu slabs
D_CH = 512                     # d-width of phase-B output chunks

_CACHE = {}


def _build_module(n_repeat=1):
    import concourse.tile as tile
    from concourse import bacc, mybir

    F32 = mybir.dt.float32
    F32R = mybir.dt.float32r
    AF = mybir.ActivationFunctionType

    nc = bacc.Bacc("TRN2", target_bir_lowering=False, debug=False)

    xt = nc.dram_tensor("xt", [D, CAP], F32R, kind="ExternalInput").ap()
    wg = nc.dram_tensor("wg", [D, F], F32R, kind="ExternalInput").ap()
    wu = nc.dram_tensor("wu", [D, F], F32R, kind="ExternalInput").ap()
    wd = nc.dram_tensor("wd", [F, D], F32R, kind="ExternalInput").ap()
    wt = nc.dram_tensor("wt", [128, CAP // 128], F32, kind="ExternalInput").ap()
    contrib = nc.dram_tensor("contrib", [CAP, D], F32, kind="ExternalOutput").ap()

    xt_r = xt.rearrange("(ko p) c -> p ko c", p=128)        # [128, 8, 2048]
    wg_r = wg.rearrange("(ko p) f -> p ko f", p=128)        # [128, 8, 4096]
    wu_r = wu.rearrange("(ko p) f -> p ko f", p=128)
    wd_r = wd.rearrange("(ko p) d -> p ko d", p=128)        # [128, 32, 1024]
    c_r = contrib.rearrange("(mo p) d -> p mo d", p=128)    # [128, 16, 1024]

    with tile.TileContext(nc) as tc:
        with (
            tc.tile_pool(name="xp", bufs=1) as xpool,
            tc.tile_pool(name="wp", bufs=2) as wpool,
            tc.tile_pool(name="hp", bufs=1) as hpool,
            tc.tile_pool(name="ep", bufs=1) as epool,
            tc.tile_pool(name="sp", bufs=2) as spool,
            tc.tile_pool(name="cp", bufs=1) as cpool,
            tc.tile_pool(name="ps", bufs=2, space="PSUM") as pspool,
        ):
            wt_t = cpool.tile([128, CAP // 128], F32)
            nc.sync.dma_start(wt_t[:], wt)

            def _emit_body():
                for cc in range(N_CC):
                    xt_t = xpool.tile([128, KD, CC_SIZE], F32R, tag="xt")
                    # per-(k, half) loads so the first matmuls wait on ~256KB
                    # instead of the whole 4MB chunk
                    for k in range(KD):
                        for nh in range(2):
                            nsl = slice(nh * (CC_SIZE // 2), (nh + 1) * (CC_SIZE // 2))
                            nc.sync.dma_start(
                                xt_t[:, k, nsl],
                                xt_r[:, k, cc * CC_SIZE + nh * (CC_SIZE // 2) :
                                     cc * CC_SIZE + (nh + 1) * (CC_SIZE // 2)],
                            )
                    eout = epool.tile([128, CC_SIZE // 128, D], F32, tag="eout")

                    for fh in range(N_FQ):
                        # ---- Phase A: gateT/upT chunks -> hid (F-major) ----
                        hid = hpool.tile([128, FQ_KF, CC_SIZE], F32R, tag="hid")
                        n_slabs = FQ_KF * 128 // W_SLAB          # 8
                        for fs in range(n_slabs):
                            f0 = fh * (FQ_KF * 128) + fs * W_SLAB
                            wg_s = wpool.tile([128, KD, W_SLAB], F32R, tag="wg")
                            wu_s = wpool.tile([128, KD, W_SLAB], F32R, tag="wu")
                            nc.sync.dma_start(wg_s[:], wg_r[:, :, f0 : f0 + W_SLAB])
                            nc.sync.dma_start(wu_s[:], wu_r[:, :, f0 : f0 + W_SLAB])
                            for fi in range(W_SLAB // 128):      # 2
                                kf_loc = fs * (W_SLAB // 128) + fi
                                for n in range(CC_SIZE // 512):  # 2
                                    pg = pspool.tile([128, 512], F32, tag="pg")
                                    pu = pspool.tile([128, 512], F32, tag="pu")
                                    for k in range(KD):
                                        nc.tensor.matmul(
                                            pg[:],
                                            lhsT=wg_s[:, k, fi * 128 : (fi + 1) * 128],
                                            rhs=xt_t[:, k, n * 512 : (n + 1) * 512],
                                            start=(k == 0),
                                            stop=(k == KD - 1),
                                        )
                                    for k in range(KD):
                                        nc.tensor.matmul(
                                            pu[:],
                                            lhsT=wu_s[:, k, fi * 128 : (fi + 1) * 128],
                                            rhs=xt_t[:, k, n * 512 : (n + 1) * 512],
                                            start=(k == 0),
                                            stop=(k == KD - 1),
                                        )
                                    sg = spool.tile([128, 512], F32, tag="sg")
                                    nc.scalar.activation(sg[:], pg[:], AF.Silu)
                                    nc.vector.tensor_mul(
                                        hid[:, kf_loc, n * 512 : (n + 1) * 512],
                                        sg[:],
                                        pu[:],
                                    )

                        # ---- Phase B (partial): eout += hidT @ Wd ----
                        for d in range(D // D_CH):               # 2
                            wd_s = wpool.tile([128, FQ_KF, D_CH], F32R, tag="wd")
                            nc.sync.dma_start(
                                wd_s[:],
                                wd_r[
                                    :,
                                    fh * FQ_KF : (fh + 1) * FQ_KF,
                                    d * D_CH : (d + 1) * D_CH,
                                ],
                            )
                            for m in range(CC_SIZE // 128):      # 8
                                gm = cc * (CC_SIZE // 128) + m
                                po = pspool.tile([128, D_CH], F32, tag="po")
                                for kf in range(FQ_KF):
                                    nc.tensor.matmul(
                                        po[:],
                                        lhsT=hid[:, kf, m * 128 : (m + 1) * 128],
                                        rhs=wd_s[:, kf, :],
                                        start=(kf == 0),
                                        stop=(kf == FQ_KF - 1),
                                    )
                                dsl = slice(d * D_CH, (d + 1) * D_CH)
                                if fh == 0:
                                    # eout = po * w   (per-token router weight)
                                    nc.vector.tensor_scalar_mul(
                                        eout[:, m, dsl], po[:], wt_t[:, gm : gm + 1]
                                    )
                                else:
                                    # eout = (po * w) + eout, then store
                                    nc.vector.scalar_tensor_tensor(
                                        eout[:, m, dsl],
                                        po[:],
                                        wt_t[:, gm : gm + 1],
                                        eout[:, m, dsl],
                                        op0=mybir.AluOpType.mult,
                                        op1=mybir.AluOpType.add,
                                    )

                    # ---- store scaled outputs ----
                    for m in range(CC_SIZE // 128):
                        gm = cc * (CC_SIZE // 128) + m
                        nc.sync.dma_start(c_r[:, gm, :], eout[:, m, :])

            if n_repeat == 1:
                _emit_body()
            else:
                with tc.For_i(0, n_repeat, 1):
                    _emit_body()

    nc.compile()
    return nc


def _get_module():
    if "nc" not in _CACHE:
        _CACHE["nc"] = _build_module()
    return _CACHE["nc"]


def _route_host(x_flat, Wr):
    """Mirror the reference's routing exactly (jax ops on CPU)."""
    import jax
    import jax.numpy as jnp

    cpu = jax.devices("cpu")[0]
    with jax.default_device(cpu):
        xj = jnp.asarray(np.asarray(x_flat, dtype=np.float32))
        wj = jnp.asarray(np.asarray(Wr, dtype=np.float32))
        scores = (xj @ wj.T).T                      # (E, N)
        top_vals, top_idx = jax.lax.top_k(scores, CAP)
        weights = jax.nn.softmax(top_vals, axis=-1)
        return np.asarray(top_idx), np.asarray(weights)


def kernel(x, Wr, Wg, Wu, Wd):
    from concourse.bass_utils import run_bass_kernel_spmd

    x = np.asarray(x, dtype=np.float32)
    Wr = np.asarray(Wr, dtype=np.float32)
    Wg = np.asarray(Wg, dtype=np.float32)
    Wu = np.asarray(Wu, dtype=np.float32)
    Wd = np.asarray(Wd, dtype=np.float32)

    x_flat = x.reshape(N, D)
    top_idx, weights = _route_host(x_flat, Wr)

    nc = _get_module()
    in_maps = []
    for e in range(N_CORES):
        chosenT = np.ascontiguousarray(x_flat[top_idx[e]].T)            # (D, CAP)
        wt_tiled = np.ascontiguousarray(
            weights[e].reshape(CAP // 128, 128).T                        # (128, 16)
        )
        in_maps.append(
            {
                "xt": chosenT,
                "wg": np.ascontiguousarray(Wg[e]),
                "wu": np.ascontiguousarray(Wu[e]),
                "wd": np.ascontiguousarray(Wd[e]),
                "wt": wt_tiled,
            }
        )

    _CACHE["last_in_maps"] = in_maps
    res = run_bass_kernel_spmd(nc, in_maps, core_ids=list(range(N_CORES)))
    _CACHE["last_results"] = res

    out = np.zeros((N, D), dtype=np.float32)
    for e in range(N_CORES):
        out[top_idx[e]] += res.results[e]["contrib"]
    aux_loss = np.asarray(0.0, dtype=np.float32)
    return out.reshape(B, S, D), aux_loss
